# revision 31
# baseline (speedup 1.0000x reference)
"""AdaptiveChebConv (K=3) distributed Bass kernel for 8 TRN2 NeuronCores.

Data-parallel over batch: B=16 -> 2 batches per core. adj/Theta replicated.

Per-core algorithm (per local batch b; N=1024, F=O=64, T=12):

  out = relu(W0 + A^T (W1 + A^T W2)),   W_k[n,o,t] = sum_f X[n,f,t] Theta_k[f,o]

v2: t-pair packed Theta matmuls. The host stacks X^T for two adjacent
timesteps into the 128 partition rows of each stationary tile
(xt2[b,nt,tp] rows = (u,f), u = t parity), and Theta is expanded to a
block-diagonal [128,128] operand per k. One LDWEIGHTS then streams 128
columns of (t,o)-major output — half the stationary loads and 2x the
work per load vs one-t-per-matmul. These matmuls drop directly into the
hop PSUM groups (start=True prologue), so only W2 and V are ever
materialized in SBUF. Output is stored bf16 and upcast on host.
"""
import sys

if "/opt/trn_rl_repo" not in sys.path:
    sys.path.insert(0, "/opt/trn_rl_repo")

import numpy as np
from contextlib import ExitStack

import concourse.bass as bass
import concourse.tile as tile
from concourse import bacc, mybir
from concourse.bass_utils import run_bass_kernel_spmd

N_CORES = 8
B, N, F, T, K, O = 16, 1024, 64, 12, 3, 64
BL = B // N_CORES          # local batches per core = 2
NT = N // 128              # n-tiles = 8
FT = F * T                 # 768
OT = O * T                 # 768

F32 = mybir.dt.float32
BF16 = mybir.dt.bfloat16

_NC = None


class Ctx:
    pass


def _emit_theta(cx, b, nt, kk, pz, ch, stop):
    """3 t-pair theta matmuls: pz[:, j*128:+128] = X_tp^T @ Theta2_kk."""
    nc = cx.nc
    for j in range(3):
        tp = ch * 3 + j
        nc.tensor.matmul(
            pz[:, j * 128:(j + 1) * 128],
            cx.xt2_t[(b, nt)][:, tp * 128:(tp + 1) * 128],
            cx.theta2_t[:, kk * 128:(kk + 1) * 128],
            start=(j == 0),
            stop=(stop and j == 2),
        )


def _emit_w21(cx, b, W2, W1):
    """W2 = X.Theta_2 and W1 = X.Theta_1 in one pass over the X^T tiles.

    Each stationary load feeds two matmuls (k=2 then k=1 with the
    LDWEIGHTS elided on the second), halving the weight-load traffic
    that otherwise dominates these 128-column matmuls.
    """
    nc = cx.nc
    for nt in range(NT):
        pzs = []
        for ch in range(2):
            pz2 = cx.zp.tile([128, 384], F32, tag="zp", name="pq2")
            pz1 = cx.zp.tile([128, 384], F32, tag="zp", name="pq1")
            for j in range(3):
                tp = ch * 3 + j
                xsl = cx.xt2_t[(b, nt)][:, tp * 128:(tp + 1) * 128]
                nc.tensor.matmul(
                    pz2[:, j * 128:(j + 1) * 128], xsl,
                    cx.theta2_t[:, 2 * 128:3 * 128],
                    start=(j == 0), stop=(j == 2),
                )
                m = nc.tensor.matmul(
                    pz1[:, j * 128:(j + 1) * 128], xsl,
                    cx.theta2_t[:, 1 * 128:2 * 128],
                    start=(j == 0), stop=(j == 2),
                )
                m.ins.ldweights = False  # reuse the stationary just loaded
            pzs.append((pz2, pz1))
        for ch in range(2):
            s = slice(nt * FT + ch * 384, nt * FT + (ch + 1) * 384)
            nc.vector.tensor_copy(W2[:, s], pzs[ch][0])
            nc.vector.tensor_copy(W1[:, s], pzs[ch][1])


def _emit_hop(cx, b, A_t, rhs_all, kk, dst, W_add=None, groups=((0, 4), (4, 4))):
    """dst = A^T rhs_all (+ X.Theta_kk prologue, or + W_add epilogue).

    Processes nt in groups with the mt loop outermost so the PSUM chains
    consume A tiles as the DMA delivers them. If dst is None this is the
    output stage: relu-copy ((t,o)-major, contiguous) + DMA.
    """
    nc = cx.nc
    for start_nt, width in groups:
        nts = range(start_nt, start_nt + width)
        pz = {}
        for nt in nts:
            for ch in range(2):
                p = cx.zp.tile([128, 384], F32, tag="zp", name="pz")
                if kk is not None:
                    _emit_theta(cx, b, nt, kk, p, ch, False)
                pz[(nt, ch)] = p
        for mt in range(NT):
            for nt in nts:
                for ch in range(2):
                    nc.tensor.matmul(
                        pz[(nt, ch)][:],
                        A_t[mt][:, nt * 128:(nt + 1) * 128],
                        rhs_all[:, mt * FT + ch * 384: mt * FT + ch * 384 + 384],
                        start=(kk is None and mt == 0),
                        stop=(mt == NT - 1),
                    )
        for nt in nts:
            o_tile = None
            if dst is None:
                o_tile = cx.out_pool.tile(
                    [128, OT], BF16, tag="out", name="o_tile"
                )
            for ch in range(2):
                p = pz[(nt, ch)]
                s = slice(nt * FT + ch * 384, nt * FT + (ch + 1) * 384)
                if dst is not None:
                    if W_add is not None:
                        nc.vector.tensor_add(dst[:, s], p[:], W_add[:, s])
                    else:
                        nc.vector.tensor_copy(dst[:, s], p[:])
                    continue
                if ch == 0:
                    nc.vector.tensor_relu(o_tile[:, 0:384], p[:])
                else:
                    nc.scalar.activation(
                        o_tile[:, 384:768], p[:],
                        mybir.ActivationFunctionType.Relu,
                    )
                nc.sync.dma_start(
                    cx.out_ap[
                        b, nt * 128:(nt + 1) * 128, ch * 6:(ch + 1) * 6, :
                    ].rearrange("p t o -> p (t o)"),
                    o_tile[:, ch * 384:(ch + 1) * 384],
                )


def _build():
    nc = bacc.Bacc("TRN2", target_bir_lowering=False, debug=False)
    cx = Ctx()
    cx.nc = nc
    cx.xt2_ap = nc.dram_tensor(
        "xt2", [BL, NT, 128, 6, 128], BF16, kind="ExternalInput"
    ).ap()
    cx.a_ap = nc.dram_tensor("a_mat", [BL, N, N], BF16, kind="ExternalInput").ap()
    cx.theta2_ap = nc.dram_tensor(
        "theta2", [128, K * 128], BF16, kind="ExternalInput"
    ).ap()
    cx.out_ap = nc.dram_tensor("out", [BL, N, T, O], BF16, kind="ExternalOutput").ap()

    with tile.TileContext(nc) as tc, ExitStack() as ctx:
        cx.a_pool = ctx.enter_context(tc.tile_pool(name="apool", bufs=2))
        cx.w_pool = ctx.enter_context(tc.tile_pool(name="wpool", bufs=2))
        cx.xt_pool = ctx.enter_context(tc.tile_pool(name="xtpool", bufs=1))
        cx.out_pool = ctx.enter_context(tc.tile_pool(name="outp", bufs=4))
        cx.const_pool = ctx.enter_context(tc.tile_pool(name="const", bufs=1))
        cx.zp = ctx.enter_context(tc.tile_pool(name="zp", bufs=8, space="PSUM"))

        cx.theta2_t = cx.const_pool.tile([128, K * 128], BF16, tag="theta2")
        nc.sync.dma_start(cx.theta2_t[:], cx.theta2_ap)
        cx.dma_dep = None

        # HAM warmup: dummy matmuls gated only on an on-device memset (no
        # DMA dependency -- input DMA data doesn't land until ~9us). They
        # fill the DMA-startup window with PE activity so the clock gate
        # reaches 8/8 before the real work begins.
        wsrc = cx.const_pool.tile([128, 384], BF16, tag="wsrc")
        nc.vector.memset(wsrc[:], 0.0)
        wz = cx.zp.tile([128, 384], F32, tag="zp", name="warm")
        for _ in range(8):
            nc.tensor.matmul(
                wz[:], wsrc[:, 0:128], wsrc[:, 0:384], start=True, stop=True,
            )
        # xt2 tiles: host-packed t-pair X^T per (batch, n-tile): [128, 6*128]
        cx.xt2_t = {}
        for b in range(BL):
            for nt in range(NT):
                cx.xt2_t[(b, nt)] = cx.xt_pool.tile(
                    [128, 6 * 128], BF16, tag=f"xt2_{nt}_{b}", name=f"X{b}_{nt}"
                )

        # DMA phases per batch: all xt2(b) at full rate (paces W2's theta
        # matmuls with no gaps, keeping HAM warm), then all A(b) at full
        # rate (paces the hop chains). The tensor queue runs
        # W2(b)->hop1(b)->hop2(b) per batch so batch b+1's DMA-paced W2
        # matmuls never sit in front of batch b's ready chains.
        for b in range(BL):
            for i in range(NT):
                d = nc.sync.dma_start(
                    cx.xt2_t[(b, i)][:],
                    cx.xt2_ap[b, i].rearrange("p tp n -> p (tp n)"),
                )
                if cx.dma_dep is not None:
                    bass._add_dep_helper(
                        d.ins, cx.dma_dep.ins, True, "dma phase order"
                    )
            A_t = []
            for i in range(NT):
                a = cx.a_pool.tile(
                    [128, 1024], BF16, tag=f"A{i}", bufs=2, name=f"A{b}_{i}"
                )
                last = nc.sync.dma_start(
                    a[:], cx.a_ap[b, i * 128:(i + 1) * 128, :]
                )
                if cx.dma_dep is not None:
                    bass._add_dep_helper(
                        last.ins, cx.dma_dep.ins, True, "dma phase order"
                    )
                A_t.append(a)
            cx.dma_dep = last  # gate next batch's DMAs behind this batch

            W2 = cx.w_pool.tile([128, NT * FT], BF16, tag="W2", name=f"W2_{b}")
            W1 = cx.w_pool.tile([128, NT * FT], BF16, tag="W1", name=f"W1_{b}")
            _emit_w21(cx, b, W2, W1)
            V = cx.w_pool.tile([128, NT * FT], BF16, tag="V", name=f"V_{b}")
            _emit_hop(cx, b, A_t, W2, None, V, W_add=W1)
            _emit_hop(
                cx, b, A_t, V, 0, None,
                groups=((0, 4), (4, 2), (6, 2)) if b == BL - 1
                else ((0, 4), (4, 4)),
            )

    nc.compile()
    return nc


def _make_in_maps(inputs):
    import ml_dtypes

    bf = ml_dtypes.bfloat16
    x = np.asarray(inputs["x"], dtype=np.float32).astype(bf)
    # [B, N, F, T] -> [B, NT, (u f), 6, n]: two timesteps stacked per tile,
    # partition dim (u,f) leading so the device DMA is a plain flatten
    x5 = x.reshape(B, NT, 128, F, T).transpose(0, 1, 4, 3, 2)  # [B,NT,T,F,128]
    xt2 = np.ascontiguousarray(
        x5.reshape(B, NT, 6, 2, F, 128).transpose(0, 1, 3, 4, 2, 5)
        .reshape(B, NT, 128, 6, 128)
    )
    # host pre-multiplies A = adj * attn: kills the on-device elementwise
    # mul, the adj DMA (2MB/core), and makes A tiles pure DMA arrivals
    a_mat = np.ascontiguousarray(
        (
            np.asarray(inputs["adj"], dtype=np.float32)[None, :, :]
            * np.asarray(inputs["spatial_attention"], dtype=np.float32)
        ).astype(bf)
    )
    th = np.asarray(inputs["Theta"], dtype=np.float32).astype(bf)
    theta2 = np.zeros((128, K, 128), dtype=bf)
    for k in range(K):
        theta2[:F, k, :O] = th[k]
        theta2[F:, k, O:] = th[k]
    theta2 = np.ascontiguousarray(theta2.reshape(128, K * 128))

    in_maps = []
    for i in range(N_CORES):
        s = slice(i * BL, (i + 1) * BL)
        in_maps.append(
            {
                "xt2": xt2[s],
                "a_mat": a_mat[s],
                "theta2": theta2,
            }
        )
    return in_maps


def kernel(**inputs):
    global _NC
    if _NC is None:
        _NC = _build()
    nc = _NC
    in_maps = _make_in_maps(inputs)
    res = run_bass_kernel_spmd(nc, in_maps, core_ids=list(range(N_CORES)))
    out = np.concatenate([res.results[i]["out"] for i in range(N_CORES)], axis=0)
    # device emits [B, N, T, O] bf16; harness expects [B, N, O, T] f32
    return out.transpose(0, 1, 3, 2).astype(np.float32)


# revision 34
# speedup vs baseline: 1.0861x; 1.0861x over previous
"""AdaptiveChebConv (K=3) distributed Bass kernel for 8 TRN2 NeuronCores.

Data-parallel over batch: B=16 -> 2 batches per core. adj/Theta replicated.

Per-core algorithm (per local batch b; N=1024, F=O=64, T=12):

  out = relu(W0 + A^T (W1 + A^T W2)),   W_k[n,o,t] = sum_f X[n,f,t] Theta_k[f,o]

v2: t-pair packed Theta matmuls. The host stacks X^T for two adjacent
timesteps into the 128 partition rows of each stationary tile
(xt2[b,nt,tp] rows = (u,f), u = t parity), and Theta is expanded to a
block-diagonal [128,128] operand per k. One LDWEIGHTS then streams 128
columns of (t,o)-major output — half the stationary loads and 2x the
work per load vs one-t-per-matmul. These matmuls drop directly into the
hop PSUM groups (start=True prologue), so only W2 and V are ever
materialized in SBUF. Output is stored bf16 and upcast on host.
"""
import sys

if "/opt/trn_rl_repo" not in sys.path:
    sys.path.insert(0, "/opt/trn_rl_repo")

import numpy as np
from contextlib import ExitStack

import concourse.bass as bass
import concourse.tile as tile
from concourse import bacc, mybir
from concourse.bass_utils import run_bass_kernel_spmd

N_CORES = 8
B, N, F, T, K, O = 16, 1024, 64, 12, 3, 64
BL = B // N_CORES          # local batches per core = 2
NT = N // 128              # n-tiles = 8
FT = F * T                 # 768
OT = O * T                 # 768

F32 = mybir.dt.float32
BF16 = mybir.dt.bfloat16

_NC = None


class Ctx:
    pass


def _emit_theta(cx, b, nt, kk, pz, ch, stop):
    """3 t-pair theta matmuls: pz[:, j*128:+128] = X_tp^T @ Theta2_kk."""
    nc = cx.nc
    for j in range(3):
        tp = ch * 3 + j
        nc.tensor.matmul(
            pz[:, j * 128:(j + 1) * 128],
            cx.xt2_t[(b, nt)][:, tp * 128:(tp + 1) * 128],
            cx.theta2_t[:, kk * 128:(kk + 1) * 128],
            start=(j == 0),
            stop=(stop and j == 2),
        )


def _emit_w2(cx, b, W2):
    """W2 = X . Theta_2 in (t,o)-major layout via t-pair matmuls."""
    nc = cx.nc
    for nt in range(NT):
        pzs = []
        for ch in range(2):
            pz = cx.zp.tile([128, 384], F32, tag="zp", name="pq")
            _emit_theta(cx, b, nt, 2, pz, ch, True)
            pzs.append(pz)
        for ch in range(2):
            nc.vector.tensor_copy(
                W2[:, nt * FT + ch * 384: nt * FT + (ch + 1) * 384], pzs[ch]
            )


def _emit_hop(cx, b, A_t, rhs_all, kk, dst, W_add=None, groups=((0, 4), (4, 4))):
    """dst = A^T rhs_all (+ X.Theta_kk prologue, or + W_add epilogue).

    Processes nt in groups with the mt loop outermost so the PSUM chains
    consume A tiles as the DMA delivers them. If dst is None this is the
    output stage: relu-copy ((t,o)-major, contiguous) + DMA.
    """
    nc = cx.nc
    for start_nt, width in groups:
        nts = range(start_nt, start_nt + width)
        pz = {}
        for nt in nts:
            for ch in range(2):
                p = cx.zp.tile([128, 384], F32, tag="zp", name="pz")
                if kk is not None:
                    _emit_theta(cx, b, nt, kk, p, ch, False)
                pz[(nt, ch)] = p
        for mt in range(NT):
            for nt in nts:
                for ch in range(2):
                    nc.tensor.matmul(
                        pz[(nt, ch)][:],
                        A_t[mt][:, nt * 128:(nt + 1) * 128],
                        rhs_all[:, mt * FT + ch * 384: mt * FT + ch * 384 + 384],
                        start=(kk is None and mt == 0),
                        stop=(mt == NT - 1),
                    )
        for nt in nts:
            o_tile = None
            if dst is None:
                o_tile = cx.out_pool.tile(
                    [128, OT], BF16, tag="out", name="o_tile"
                )
            for ch in range(2):
                p = pz[(nt, ch)]
                s = slice(nt * FT + ch * 384, nt * FT + (ch + 1) * 384)
                if dst is not None:
                    if W_add is not None:
                        nc.vector.tensor_add(dst[:, s], p[:], W_add[:, s])
                    else:
                        nc.vector.tensor_copy(dst[:, s], p[:])
                    continue
                if ch == 0:
                    nc.vector.tensor_relu(o_tile[:, 0:384], p[:])
                else:
                    nc.scalar.activation(
                        o_tile[:, 384:768], p[:],
                        mybir.ActivationFunctionType.Relu,
                    )
                nc.sync.dma_start(
                    cx.out_ap[
                        b, nt * 128:(nt + 1) * 128, ch * 6:(ch + 1) * 6, :
                    ].rearrange("p t o -> p (t o)"),
                    o_tile[:, ch * 384:(ch + 1) * 384],
                )


def _build():
    nc = bacc.Bacc("TRN2", target_bir_lowering=False, debug=False)
    cx = Ctx()
    cx.nc = nc
    cx.xt2_ap = nc.dram_tensor(
        "xt2", [BL, NT, 128, 6, 128], BF16, kind="ExternalInput"
    ).ap()
    cx.a_ap = nc.dram_tensor("a_mat", [BL, N, N], BF16, kind="ExternalInput").ap()
    cx.theta2_ap = nc.dram_tensor(
        "theta2", [128, K * 128], BF16, kind="ExternalInput"
    ).ap()
    cx.out_ap = nc.dram_tensor("out", [BL, N, T, O], BF16, kind="ExternalOutput").ap()

    with tile.TileContext(nc) as tc, ExitStack() as ctx:
        cx.a_pool = ctx.enter_context(tc.tile_pool(name="apool", bufs=2))
        cx.w_pool = ctx.enter_context(tc.tile_pool(name="wpool", bufs=2))
        cx.xt_pool = ctx.enter_context(tc.tile_pool(name="xtpool", bufs=1))
        cx.out_pool = ctx.enter_context(tc.tile_pool(name="outp", bufs=4))
        cx.const_pool = ctx.enter_context(tc.tile_pool(name="const", bufs=1))
        cx.zp = ctx.enter_context(tc.tile_pool(name="zp", bufs=8, space="PSUM"))

        cx.theta2_t = cx.const_pool.tile([128, K * 128], BF16, tag="theta2")
        nc.sync.dma_start(cx.theta2_t[:], cx.theta2_ap)
        cx.dma_dep = None

        # HAM warmup: dummy matmuls gated only on an on-device memset (no
        # DMA dependency -- input DMA data doesn't land until ~9us). They
        # fill the DMA-startup window with PE activity so the clock gate
        # reaches 8/8 before the real work begins.
        wsrc = cx.const_pool.tile([128, 384], BF16, tag="wsrc")
        nc.vector.memset(wsrc[:], 0.0)
        wz = cx.zp.tile([128, 384], F32, tag="zp", name="warm")
        for _ in range(12):
            nc.tensor.matmul(
                wz[:], wsrc[:, 0:128], wsrc[:, 0:384], start=True, stop=True,
            )
        # xt2 tiles: host-packed t-pair X^T per (batch, n-tile): [128, 6*128]
        cx.xt2_t = {}
        for b in range(BL):
            for nt in range(NT):
                cx.xt2_t[(b, nt)] = cx.xt_pool.tile(
                    [128, 6 * 128], BF16, tag=f"xt2_{nt}_{b}", name=f"X{b}_{nt}"
                )

        # DMA phases per batch: all xt2(b) at full rate (paces W2's theta
        # matmuls with no gaps, keeping HAM warm), then all A(b) at full
        # rate (paces the hop chains). The tensor queue runs
        # W2(b)->hop1(b)->hop2(b) per batch so batch b+1's DMA-paced W2
        # matmuls never sit in front of batch b's ready chains.
        for b in range(BL):
            for i in range(NT):
                d = nc.sync.dma_start(
                    cx.xt2_t[(b, i)][:],
                    cx.xt2_ap[b, i].rearrange("p tp n -> p (tp n)"),
                )
                if cx.dma_dep is not None:
                    bass._add_dep_helper(
                        d.ins, cx.dma_dep.ins, True, "dma phase order"
                    )
            A_t = []
            for i in range(NT):
                a = cx.a_pool.tile(
                    [128, 1024], BF16, tag=f"A{i}", bufs=2, name=f"A{b}_{i}"
                )
                last = nc.sync.dma_start(
                    a[:], cx.a_ap[b, i * 128:(i + 1) * 128, :]
                )
                if cx.dma_dep is not None:
                    bass._add_dep_helper(
                        last.ins, cx.dma_dep.ins, True, "dma phase order"
                    )
                A_t.append(a)
            cx.dma_dep = last  # gate next batch's DMAs behind this batch

            W2 = cx.w_pool.tile([128, NT * FT], BF16, tag="W2", name=f"W2_{b}")
            _emit_w2(cx, b, W2)
            V = cx.w_pool.tile([128, NT * FT], BF16, tag="V", name=f"V_{b}")
            _emit_hop(cx, b, A_t, W2, 1, V)
            _emit_hop(
                cx, b, A_t, V, 0, None,
                groups=((0, 4), (4, 2), (6, 2)) if b == BL - 1
                else ((0, 4), (4, 4)),
            )

    nc.compile()
    return nc


def _make_in_maps(inputs):
    import ml_dtypes

    bf = ml_dtypes.bfloat16
    x = np.asarray(inputs["x"], dtype=np.float32).astype(bf)
    # [B, N, F, T] -> [B, NT, (u f), 6, n]: two timesteps stacked per tile,
    # partition dim (u,f) leading so the device DMA is a plain flatten
    x5 = x.reshape(B, NT, 128, F, T).transpose(0, 1, 4, 3, 2)  # [B,NT,T,F,128]
    xt2 = np.ascontiguousarray(
        x5.reshape(B, NT, 6, 2, F, 128).transpose(0, 1, 3, 4, 2, 5)
        .reshape(B, NT, 128, 6, 128)
    )
    # host pre-multiplies A = adj * attn: kills the on-device elementwise
    # mul, the adj DMA (2MB/core), and makes A tiles pure DMA arrivals
    a_mat = np.ascontiguousarray(
        (
            np.asarray(inputs["adj"], dtype=np.float32)[None, :, :]
            * np.asarray(inputs["spatial_attention"], dtype=np.float32)
        ).astype(bf)
    )
    th = np.asarray(inputs["Theta"], dtype=np.float32).astype(bf)
    theta2 = np.zeros((128, K, 128), dtype=bf)
    for k in range(K):
        theta2[:F, k, :O] = th[k]
        theta2[F:, k, O:] = th[k]
    theta2 = np.ascontiguousarray(theta2.reshape(128, K * 128))

    in_maps = []
    for i in range(N_CORES):
        s = slice(i * BL, (i + 1) * BL)
        in_maps.append(
            {
                "xt2": xt2[s],
                "a_mat": a_mat[s],
                "theta2": theta2,
            }
        )
    return in_maps


def kernel(**inputs):
    global _NC
    if _NC is None:
        _NC = _build()
    nc = _NC
    in_maps = _make_in_maps(inputs)
    res = run_bass_kernel_spmd(nc, in_maps, core_ids=list(range(N_CORES)))
    out = np.concatenate([res.results[i]["out"] for i in range(N_CORES)], axis=0)
    # device emits [B, N, T, O] bf16; harness expects [B, N, O, T] f32
    return out.transpose(0, 1, 3, 2).astype(np.float32)
